# revision 18
# baseline (speedup 1.0000x reference)
# CapsuleNetwork Trainium2 kernel (8-core data parallel, 4 images/core).
#
# Per core:
#   conv1 3->256 k9 s1: fp8 DoubleRow im2col matmuls (K=244 padded), relu
#     fused into drains, output h1 in fp8 at x32 scale.
#   conv2 256->256 k9 s2: fp8 DoubleRow matmuls (K=256/instr, 157 TF/s).
#     w2 quantized to fp8e4 with error-feedback rounding per (o,i,parity
#     class) so tap-sum quantization error cancels in the routing pixel sums.
#   capsule squash (pixel-major via PE transposes), dynamic routing (3 iters)
#   per image, without materializing u_hat:
#     F'[(r,i),(r',c)] = sum_p capsW[p,(r,i)] * exp(b)[p,(r',c)]; G = diag blocks
#     s[c,o]  = sum_{r,i} G[r,c,i] * route_w[r,c,i,o]   (per-class matmuls)
#     b     += caps @ T_block,  T_block[(r,i),(r,c)] = sum_o w[r,c,i,o] v[c,o]
# Emission interleaves per-image stages so conv1(b+1), squash(b-1) and the
# routing iterations of finished images all hide inside the conv2 matmul
# stream; only image 3's routing chain remains as a short tail.
import functools
from contextlib import ExitStack

import numpy as np
import ml_dtypes

import concourse.bass as bass
import concourse.tile as tile
from concourse import bacc
from concourse import mybir
from concourse.bass_utils import run_bass_kernel_spmd

BF = mybir.dt.bfloat16
F8 = mybir.dt.float8e4
F32 = mybir.dt.float32
AF = mybir.ActivationFunctionType
AX = mybir.AxisListType
DR = mybir.MatmulPerfMode.DoubleRow

NCORES = 8
B = 4              # images per core
K1 = 243           # 3*9*9 im2col contraction
KP = 122           # DoubleRow K-pairs (244 = 2*122, one zero pad row)
NPIX1 = 3136       # 56*56 conv1 output pixels
N1CH = 448         # conv1 moving chunk (3136 = 7*448)
PIX = 576          # 24*24 conv2 output pixels
PIX_CHUNKS = [(0, 128), (128, 128), (256, 128), (384, 128), (512, 64)]
R, D, C, O = 32, 8, 10, 16
XS = 8.0           # x fp8 scale
WS1 = 4.0          # w1 fp8 scale  (XS*WS1 == HS so drains need no rescale)
HS = 32.0          # h1 fp8 scale
WS2 = 1024.0       # w2 fp8 scale
DS = 1.0 / (HS * WS2)  # conv2 psum descale
NTG = 9            # w2 dma tap groups (81 = 9*9)


def _build_nc():
    nc = bacc.Bacc("TRN2", target_bir_lowering=False, debug=False)
    # register the squash-eps constant for activation bias use
    eps_t = nc.alloc_sbuf_tensor("const-eps", [128, 1], F32)
    nc.gpsimd.memset(eps_t.ap(), 1e-8)
    nc.const_aps.aps[(F32, 1e-8)] = eps_t.ap()
    nc.all_engine_barrier()
    x_d = nc.declare_dram_parameter("x", [B, KP, 2, NPIX1], F8, isOutput=False)
    w1_d = nc.declare_dram_parameter("w1", [KP, 2, 256], F8, isOutput=False)
    b1_d = nc.declare_dram_parameter("b1", [256, 1], F32, isOutput=False)
    w2_d = nc.declare_dram_parameter("w2", [2, 128, 81, 2, 128], F8, isOutput=False)
    b2_d = nc.declare_dram_parameter("b2", [256, 1], F32, isOutput=False)
    ws_d = nc.declare_dram_parameter("ws", [256, C * O], BF, isOutput=False)
    wcob_d = nc.declare_dram_parameter("wcob", [O, C, 256], BF, isOutput=False)
    maskg_d = nc.declare_dram_parameter("maskg", [2, 128, R * C], F32, isOutput=False)
    idf_d = nc.declare_dram_parameter("idf", [128, 128], F32, isOutput=False)
    idb_d = nc.declare_dram_parameter("idb", [128, 128], BF, isOutput=False)
    vout_d = nc.declare_dram_parameter("v_out", [B * C, O], F32, isOutput=True)

    with tile.TileContext(nc) as tc, ExitStack() as ctx:
        consts = ctx.enter_context(tc.tile_pool(name="consts", bufs=1))
        w18 = consts.tile([KP, 2, 256], F8, tag="w18", name="w18")
        nc.gpsimd.dma_start(w18, w1_d[:, :, :])
        b1t = [consts.tile([128, 1], F32, tag=f"b1_{m}", name=f"b1_{m}") for m in range(2)]
        b2t = [consts.tile([128, 1], F32, tag=f"b2_{m}", name=f"b2_{m}") for m in range(2)]
        ws_t = [consts.tile([128, C * O], BF, tag=f"ws{m}", name=f"ws{m}") for m in range(2)]
        wcob = consts.tile([O, C, 256], BF, tag="wcob", name="wcob")
        idf = consts.tile([128, 128], F32, tag="idf", name="idf")
        idb = consts.tile([128, 128], BF, tag="idb", name="idb")
        maskg = [consts.tile([128, R * C], F32, tag=f"mg{m}", name=f"mg{m}")
                 for m in range(2)]
        ones16 = consts.tile([16, 1], F32, tag="ones16", name="ones16")
        ones1 = consts.tile([1, 16], F32, tag="ones1", name="ones1")
        nc.vector.memset(ones16, 1.0)
        nc.vector.memset(ones1, 1.0)

        # ---- persistent tensors ----
        persist = ctx.enter_context(tc.tile_pool(name="persist", bufs=1))
        caps_bf = [persist.tile([128, B, 256], BF, tag=f"cbf{k}", name=f"cbf{k}")
                   for k in range(5)]                              # pixel-major squashed
        capsT = [[persist.tile([128, PIX], BF, tag=f"cT{b}_{g}", name=f"cT{b}_{g}")
                  for g in range(2)] for b in range(B)]            # channel-major squashed
        capsum = [persist.tile([128, B], F32, tag=f"cs{g}", name=f"cs{g}")
                  for g in range(2)]
        capsum_bf = [persist.tile([128, B], BF, tag=f"csb{g}", name=f"csb{g}")
                     for g in range(2)]
        # fp8 conv1 output, both ig halves in one tile for DoubleRow rhs:
        # [ki, ig, b, y(56), phase(2), x'(28)]
        h18 = persist.tile([128, 2, B, 56, 2, 28], F8, tag="h18", name="h18")
        # fp8 conv2 weights: [ki, tap, ig, mo] per og half
        w2t = [persist.tile([128, 81, 2, 128], F8, tag=f"w2_{og}", name=f"w2_{og}")
               for og in range(2)]
        # im2col inputs, all 4 images resident (fp8, 6.3KB/partition each)
        impool = ctx.enter_context(tc.tile_pool(name="imcol", bufs=1))
        imt = [impool.tile([KP, 2, NPIX1], F8, tag=f"im{b}", name=f"im{b}")
               for b in range(B)]
        # first two images split across 4 queue families so they land in a
        # few us; later images ride whole on the hw queues (plenty of slack)
        QS = [(nc.gpsimd, 0, 41), (nc.sync, 41, 81), (nc.scalar, 81, KP)]
        for b in (0, 1):
            for eng, p0, p1 in QS:
                eng.dma_start(imt[b][p0:p1], x_d[b, p0:p1])
        for m in range(2):  # drain biases, needed ~8us in
            nc.gpsimd.dma_start(b1t[m], b1_d[m * 128:(m + 1) * 128, :])
            nc.gpsimd.dma_start(b2t[m], b2_d[m * 128:(m + 1) * 128, :])
        TG = 81 // NTG
        for og in range(2):   # tap-group granular so conv2 starts early
            for g in range(NTG):
                nc.gpsimd.dma_start(w2t[og][:, g * TG:(g + 1) * TG],
                                    w2_d[og, :, g * TG:(g + 1) * TG])
            if og == 0:
                nc.gpsimd.dma_start(idb, idb_d[:, :])
        nc.sync.dma_start(imt[2], x_d[2])
        nc.scalar.dma_start(imt[3], x_d[3])
        for m in range(2):
            nc.gpsimd.dma_start(maskg[m], maskg_d[m])
            nc.gpsimd.dma_start(ws_t[m], ws_d[m * 128:(m + 1) * 128, :])
        nc.gpsimd.dma_start(wcob, wcob_d[:, :, :])
        nc.gpsimd.dma_start(idf, idf_d[:, :])

        rpool = ctx.enter_context(tc.tile_pool(name="rpool", bufs=1))
        dpool = ctx.enter_context(tc.tile_pool(name="dtmp", bufs=4))
        blog = [rpool.tile([128, B, R, C], F32, tag=f"bl{k}", name=f"bl{k}")
                for k in range(5)]
        crawpool = ctx.enter_context(tc.tile_pool(name="craw", bufs=1))
        capsT_raw = [[crawpool.tile([128, PIX], BF, tag=f"cr{b}_{g}", name=f"cr{b}_{g}")
                      for g in range(2)] for b in range(B)]

        with tc.tile_pool(name="c1psum", bufs=2, space="PSUM") as c1psum, \
             tc.tile_pool(name="c2psum", bufs=2, space="PSUM") as c2psum, \
             tc.tile_pool(name="ttpsum", bufs=2, space="PSUM") as ttpsum, \
             tc.tile_pool(name="rsp", bufs=2, space="PSUM") as rsp, \
             tc.tile_pool(name="pmraw", bufs=2) as pmpool, \
             tc.tile_pool(name="sqtmp", bufs=4) as sqpool:

            def rtile():
                return rsp.tile([128, R * C], F32, tag="r", name="r")

            def conv1(b):
                for m in range(2):
                    for n in range(7):  # 448 pixels = 8 rows of 56
                        ps = c1psum.tile([128, 8, 56], F32, tag="c1ps", name="c1ps")
                        nc.tensor.matmul(ps, w18[:, :, m * 128:(m + 1) * 128],
                                         imt[b][:, :, n * N1CH:(n + 1) * N1CH],
                                         start=True, stop=True, perf_mode=DR)
                        # psum is at x32 scale (XS*WS1); drain = fp8(relu(.+32*b1))
                        for px in range(2):
                            dst = h18[:, m, b, 8 * n:8 * n + 8, px, :]
                            if (n + px) % 2 == 0:
                                nc.scalar.activation(dst, ps[:, :, px::2],
                                                     AF.Relu, bias=b1t[m], scale=1.0)
                            else:
                                nc.vector.tensor_scalar(
                                    dst, ps[:, :, px::2], b1t[m], 0.0,
                                    op0=mybir.AluOpType.add,
                                    op1=mybir.AluOpType.max)

            def conv2og(b, og, tr=(0, 81), pss=None):
                if pss is None:
                    pss = [c2psum.tile([128, 288], F32, tag="c2ps", name="c2ps")
                           for _ in range(2)]
                for t81 in range(*tr):
                    kh, kw = t81 // 9, t81 % 9
                    lhsT = w2t[og][:, t81]          # [ki, 2(ig), mo]
                    for y in range(2):
                        rhs = h18[:, :, b,
                                  kh + 24 * y:kh + 24 * y + 24:2,
                                  kw % 2, kw // 2:kw // 2 + 24]
                        nc.tensor.matmul(
                            pss[y], lhsT, rhs,
                            start=(t81 == 0), stop=(t81 == 80),
                            perf_mode=DR)
                if tr[1] == 81:
                    for y in range(2):
                        nc.scalar.activation(
                            capsT_raw[b][og][:, y * 288:(y + 1) * 288], pss[y],
                            AF.Identity, bias=b2t[og], scale=DS)
                return pss

            def squash_fwd(b):
                # pixel-major transpose + squash -> caps_bf (pixel-major)
                for k, (p0, ln) in enumerate(PIX_CHUNKS):
                    pm = pmpool.tile([128, 256], BF, tag="pm", name="pm")
                    for og in range(2):
                        tp = ttpsum.tile([128, 128], BF, tag="tt", name="tt")
                        nc.tensor.transpose(tp[:ln, :],
                                            capsT_raw[b][og][:, p0:p0 + ln], idb)
                        nc.vector.tensor_copy(
                            pm[:ln, og * 128:(og + 1) * 128], tp[:ln, :])
                    pm3 = pm.rearrange("p (r i) -> p r i", i=D)
                    sq = sqpool.tile([128, R, D], F32, tag="sq", name="sq")
                    nc.scalar.activation(sq[:ln], pm3[:ln], AF.Square)
                    nsq = sqpool.tile([128, R], F32, tag="nsq", name="nsq")
                    nc.vector.reduce_sum(nsq[:ln], sq[:ln], axis=AX.X)
                    a = sqpool.tile([128, R], F32, tag="sqa", name="sqa")
                    nc.scalar.activation(a[:ln], nsq[:ln], AF.Sqrt, bias=1e-8)
                    nc.vector.scalar_tensor_tensor(
                        a[:ln], nsq[:ln], 1.0, a[:ln],
                        op0=mybir.AluOpType.add, op1=mybir.AluOpType.mult)
                    nc.vector.reciprocal(a[:ln], a[:ln])
                    nc.vector.tensor_mul(a[:ln], nsq[:ln], a[:ln])
                    cbf3 = caps_bf[k][:, b].rearrange("p (r i) -> p r i", i=D)
                    nc.gpsimd.tensor_mul(
                        cbf3[:ln], pm3[:ln],
                        a[:ln].unsqueeze(2).broadcast_to([ln, R, D]))

            def squash_bwd(b):
                # transpose squashed caps back to channel-major capsT
                for k, (p0, ln) in enumerate(PIX_CHUNKS):
                    for og in range(2):
                        tb = ttpsum.tile([128, 128], BF, tag="tt", name="tt")
                        nc.tensor.transpose(
                            tb[:, :ln],
                            caps_bf[k][:ln, b, og * 128:(og + 1) * 128],
                            idb[:ln, :ln])
                        nc.vector.tensor_copy(capsT[b][og][:, p0:p0 + ln],
                                              tb[:, :ln])
                for g in range(2):  # iter-0 uniform-coupling capsule sums
                    nc.vector.reduce_sum(capsum[g][:, b:b + 1], capsT[b][g],
                                         axis=AX.X)
                    nc.gpsimd.tensor_scalar_mul(capsum_bf[g][:, b:b + 1],
                                                capsum[g][:, b:b + 1], 1.0 / C)

            # ---------------- per-image routing ----------------
            def s_matmuls(b, rhs_pair):
                """s4T [16, C] psum slice; rhs_pair[m] is [128, >=1] per image."""
                s4T = rtile()[:16, :C]
                for c in range(C):
                    for m in range(2):
                        rhs = rhs_pair[m]
                        if rhs.shape[-1] != 1:
                            rhs = rhs[:, c:c + 1]
                        nc.tensor.matmul(s4T[:, c:c + 1],
                                         ws_t[m][:, c * 16:(c + 1) * 16],
                                         rhs, start=(m == 0), stop=(m == 1))
                return s4T

            def v_squash(b, s4T, last):
                """s4T: psum slice [16 (o), C] -> v4T bf16 [16, C] or v_out."""
                s4T_sb = dpool.tile([16, C], F32, tag="s4Tsb", name="s4Tsb")
                nc.vector.tensor_copy(s4T_sb, s4T)
                if last:
                    s4 = rtile()[:C, :16]
                    nc.tensor.transpose(s4, s4T_sb, idf[:16, :16])
                    sq = dpool.tile([C, 16], F32, tag="vsq", name="vsq")
                    nc.scalar.activation(sq, s4, AF.Square)
                    nsq = dpool.tile([C, 1], F32, tag="vnsq", name="vnsq")
                    nc.vector.reduce_sum(nsq, sq, axis=AX.X)
                    a = dpool.tile([C, 1], F32, tag="va", name="va")
                    nc.scalar.activation(a, nsq, AF.Sqrt, bias=1e-8)
                    nc.vector.scalar_tensor_tensor(
                        a, nsq, 1.0, a,
                        op0=mybir.AluOpType.add, op1=mybir.AluOpType.mult)
                    nc.vector.reciprocal(a, a)
                    nc.vector.tensor_mul(a, nsq, a)
                    vout = dpool.tile([C, 16], F32, tag="vout", name="vout")
                    nc.vector.tensor_mul(vout, s4, a.broadcast_to([C, 16]))
                    nc.sync.dma_start(vout_d[b * C:(b + 1) * C, :], vout)
                    return None
                # row-major squash: partition-reduce |s|^2 via ones-matmul,
                # broadcast the scale back via a K=1 outer-product matmul.
                sqT = dpool.tile([16, C], F32, tag="vsqT", name="vsqT")
                nc.scalar.activation(sqT, s4T, AF.Square)
                nsqr = rtile()[:1, :C]
                nc.tensor.matmul(nsqr, ones16, sqT, start=True, stop=True)
                a = dpool.tile([1, C], F32, tag="var", name="var")
                nc.scalar.activation(a, nsqr, AF.Sqrt, bias=1e-8)
                nc.vector.scalar_tensor_tensor(
                    a, nsqr, 1.0, a,
                    op0=mybir.AluOpType.add, op1=mybir.AluOpType.mult)
                nc.vector.reciprocal(a, a)
                sgr = dpool.tile([1, C], F32, tag="sgr", name="sgr")
                nc.vector.tensor_mul(sgr, nsqr, a)
                sgT = rtile()[:16, :C]
                nc.tensor.matmul(sgT, ones1, sgr, start=True, stop=True)
                v4T = dpool.tile([16, C], BF, tag="v4T", name="v4T")
                nc.vector.tensor_mul(v4T, s4T_sb, sgT)
                return v4T

            def b_update(b, v4T, it):
                """b_log[b] += caps . T_block (T = route_w . v, block-diag)."""
                T4 = [dpool.tile([128, R * C], BF, tag=f"T4_{m}", name=f"T4_{m}")
                      for m in range(2)]
                for m in range(2):
                    t4 = rtile()[:, :C]
                    for c in range(C):
                        nc.tensor.matmul(t4[:, c:c + 1],
                                         wcob[:, c, m * 128:(m + 1) * 128],
                                         v4T[:, c:c + 1], start=True, stop=True)
                    data = t4.unsqueeze(1).broadcast_to([128, R, C])
                    nc.vector.tensor_mul(
                        T4[m].rearrange("p (r c) -> p r c", c=C), data,
                        maskg[m].rearrange("p (r c) -> p r c", c=C))
                for k, (p0, ln) in enumerate(PIX_CHUNKS):
                    dl = rtile()[:, :R * C]
                    for kc in range(2):
                        nc.tensor.matmul(dl[:ln], capsT[b][kc][:, p0:p0 + ln],
                                         T4[kc], start=(kc == 0), stop=(kc == 1))
                    blk = blog[k][:ln, b].rearrange("p r c -> p (r c)")
                    if it == 0:
                        nc.scalar.activation(blk, dl[:ln], AF.Identity)
                    else:
                        nc.vector.tensor_add(blk, blk, dl[:ln])

            def softmax_G(b):
                """softmax over c folded into caps; G = diag blocks of cw.T@e."""
                F4 = [rtile()[:, :R * C] for _ in range(2)]
                for k, (p0, ln) in enumerate(PIX_CHUNKS):
                    et = dpool.tile([128, R, C], BF, tag="e", name="e", bufs=2)
                    nc.scalar.activation(et[:ln], blog[k][:ln, b], AF.Exp)
                    den = dpool.tile([128, R], F32, tag="den", name="den")
                    nc.vector.reduce_sum(den[:ln], et[:ln], axis=AX.X)
                    nc.vector.reciprocal(den[:ln], den[:ln])
                    cwt = dpool.tile([128, R, D], BF, tag="cw", name="cw", bufs=2)
                    cbf4 = caps_bf[k][:, b].rearrange("p (r i) -> p r i", i=D)
                    nc.gpsimd.tensor_mul(
                        cwt[:ln], cbf4[:ln],
                        den[:ln].unsqueeze(2).broadcast_to([ln, R, D]))
                    cwf = cwt.rearrange("p r i -> p (r i)")
                    ef = et.rearrange("p r c -> p (r c)")
                    for m in range(2):
                        nc.tensor.matmul(F4[m],
                                         cwf[:ln, m * 128:(m + 1) * 128],
                                         ef[:ln], start=(k == 0), stop=(k == 4))
                Gp = [dpool.tile([128, C], BF, tag=f"G{m}", name=f"G{m}")
                      for m in range(2)]
                for m in range(2):
                    fm = dpool.tile([128, R * C], BF, tag="fm", name="fm")
                    nc.vector.tensor_mul(fm, F4[m], maskg[m])
                    gf = dpool.tile([128, C], F32, tag="gf", name="gf")
                    nc.vector.reduce_sum(
                        gf, fm.rearrange("p (r c) -> p c r", c=C), axis=AX.X)
                    nc.gpsimd.tensor_copy(Gp[m], gf)
                return Gp

            def r_iter(b, it):
                if it == 0:
                    rhs = [capsum_bf[m][:, b:b + 1] for m in range(2)]
                else:
                    rhs = softmax_G(b)
                s4T = s_matmuls(b, rhs)
                v4T = v_squash(b, s4T, last=(it == 2))
                if it < 2:
                    b_update(b, v4T, it)

            # -------- pipelined emission schedule --------
            conv1(0)
            conv1(1)
            conv2og(0, 0)
            conv2og(0, 1)
            squash_fwd(0)
            conv1(2)
            conv2og(1, 0)
            squash_bwd(0)
            conv2og(1, 1)
            squash_fwd(1)
            conv1(3)
            r_iter(0, 0)
            conv2og(2, 0)
            squash_bwd(1)
            r_iter(0, 1)
            conv2og(2, 1)
            squash_fwd(2)
            r_iter(1, 0)
            r_iter(0, 2)
            conv2og(3, 0)
            squash_bwd(2)
            r_iter(1, 1)
            pss31 = conv2og(3, 1, tr=(0, 41))
            r_iter(2, 0)
            r_iter(1, 2)
            conv2og(3, 1, tr=(41, 81), pss=pss31)
            squash_fwd(3)
            r_iter(2, 1)
            squash_bwd(3)
            r_iter(2, 2)
            r_iter(3, 0)
            r_iter(3, 1)
            r_iter(3, 2)

    nc.compile()
    return nc


@functools.lru_cache(maxsize=1)
def _get_nc():
    return _build_nc()


_F8_GRID = None


def _f8_grid():
    global _F8_GRID
    if _F8_GRID is None:
        codes = np.arange(256, dtype=np.uint8).view(ml_dtypes.float8_e4m3)
        vals = codes.astype(np.float32)
        _F8_GRID = np.unique(np.sort(vals[np.isfinite(vals)]))
    return _F8_GRID


def _ef_round(w):
    """Error-feedback rounding to the fp8e4 grid along the last axis:
    greedily picks the rounding neighbor that keeps the running tap-sum
    quantization error near zero (kills pixel-sum-coherent error)."""
    grid = _f8_grid()
    shp = w.shape
    T = shp[-1]
    wf = w.reshape(-1, T).astype(np.float32)
    idx = np.clip(np.searchsorted(grid, wf), 1, len(grid) - 1)
    lo = grid[idx - 1]
    hi = grid[idx]
    out = np.empty_like(wf)
    acc = np.zeros(wf.shape[0], np.float32)
    for t in range(T):
        dl = acc + (lo[:, t] - wf[:, t])
        dh = acc + (hi[:, t] - wf[:, t])
        pick_h = np.abs(dh) < np.abs(dl)
        out[:, t] = np.where(pick_h, hi[:, t], lo[:, t])
        acc = np.where(pick_h, dh, dl)
    return out.reshape(shp)


def _prep_consts(conv1_w, conv1_b, conv2_w, conv2_b, route_w):
    bf = ml_dtypes.bfloat16
    f8 = ml_dtypes.float8_e4m3
    f32 = np.float32
    # conv1 weights at x4 (so XS*WS1 = HS = 32 and drains need no rescale)
    w1 = np.zeros((2 * KP, 256), f32)
    w1[:K1] = conv1_w.astype(f32).transpose(1, 2, 3, 0).reshape(K1, 256) * WS1
    w1 = w1.reshape(KP, 2, 256)
    b1 = conv1_b.astype(f32).reshape(256, 1) * HS
    # conv2 weights: scale, error-feedback round per (o, i, parity class),
    # then lay out as [og, ki, tap, ig, mo]
    w2s = conv2_w.astype(f32) * WS2                     # [o, i, 9, 9]
    w2q = np.empty_like(w2s)
    for ph in range(2):
        for pw in range(2):
            blk = w2s[:, :, ph::2, pw::2]
            nh, nw = blk.shape[2], blk.shape[3]
            w2q[:, :, ph::2, pw::2] = _ef_round(
                blk.reshape(256, 256, nh * nw)).reshape(256, 256, nh, nw)
    w2q = (w2q.reshape(2, 128, 2, 128, 81)              # [og, mo, ig, ki, t]
           .transpose(0, 3, 4, 2, 1))                   # [og, ki, t, ig, mo]
    ws = route_w.astype(f32).transpose(0, 2, 1, 3).reshape(256, C * O)
    wcob = route_w.astype(f32).transpose(3, 1, 0, 2).reshape(O, C, 256)
    maskg = np.zeros((2, 128, R * C), f32)
    for m in range(2):
        for j in range(128):
            r = m * 16 + j // D
            maskg[m, j, r * C:(r + 1) * C] = 1.0
    return {
        "w1": np.ascontiguousarray(w1).astype(f8),
        "b1": np.ascontiguousarray(b1),
        "w2": np.ascontiguousarray(w2q).astype(f8),
        "b2": np.ascontiguousarray(conv2_b.astype(f32).reshape(256, 1)),
        "ws": np.ascontiguousarray(ws).astype(bf),
        "wcob": np.ascontiguousarray(wcob).astype(bf),
        "idf": np.eye(128, dtype=f32),
        "idb": np.eye(128, dtype=f32).astype(bf),
        "maskg": maskg,
    }


def _ensure_ntff_hook():
    """The agent image's antenv lacks axon_hooks; shim it so trace=True works."""
    import sys
    import types
    try:
        from antenv import axon_hooks  # noqa: F401
        return
    except ImportError:
        pass
    mod = types.ModuleType("antenv.axon_hooks")
    _h = [None]
    mod.get_axon_ntff_profile_hook = lambda: _h[0]
    mod.set_axon_ntff_profile_hook = lambda h: _h.__setitem__(0, h)
    sys.modules["antenv.axon_hooks"] = mod
    try:
        from trn_agent_boot.trn_boot import _ntff_profile_via_ctypes
        mod.set_axon_ntff_profile_hook(
            _ntff_profile_via_ctypes("/opt/axon/libaxon_pjrt.so"))
    except Exception as e:  # degrade: trace skipped, run still works
        print(f"ntff hook shim failed: {e}")


def run(x, conv1_w, conv1_b, conv2_w, conv2_b, route_w, trace=False, cores=NCORES):
    if trace:
        _ensure_ntff_hook()
    x = np.asarray(x, np.float32)
    nb = x.shape[0]
    consts = _prep_consts(np.asarray(conv1_w), np.asarray(conv1_b),
                          np.asarray(conv2_w), np.asarray(conv2_b),
                          np.asarray(route_w))
    win = np.lib.stride_tricks.sliding_window_view(x, (9, 9), axis=(2, 3))
    xb = (win.transpose(0, 1, 4, 5, 2, 3)          # [b, c, kh, kw, y, x]
          .reshape(nb, K1, NPIX1))
    xp = np.zeros((nb, 2 * KP, NPIX1), np.float32)
    xp[:, :K1] = xb * XS
    xp = xp.reshape(nb, KP, 2, NPIX1).astype(ml_dtypes.float8_e4m3)
    assert nb == B * cores
    in_maps = []
    for cid in range(cores):
        m = dict(consts)
        m["x"] = np.ascontiguousarray(xp[cid * B:(cid + 1) * B])
        in_maps.append(m)
    res = run_bass_kernel_spmd(_get_nc(), in_maps, list(range(cores)), trace=trace)
    out = np.concatenate([r["v_out"].reshape(B, C, O) for r in res.results], axis=0)
    return out.astype(np.float32), res


def kernel(x, conv1_w, conv1_b, conv2_w, conv2_b, route_w):
    out, _ = run(x, conv1_w, conv1_b, conv2_w, conv2_b, route_w, trace=False)
    return out
